# revision 22
# baseline (speedup 1.0000x reference)
"""GQA attention (RoPE, causal) on 8 TRN2 NeuronCores.

Sharding: core c = (b, g) with b = c // 4 (batch), g = c % 4 (kv-group).
Each core computes 4 query heads + 1 kv head of attention for one batch
element, plus its slice of the output projection; the host sums the 4
partial outputs per batch (row-parallel wo unshard).

Layout tricks:
- x is shipped pre-transposed (xT [DIM, T]) so no on-device transposes;
  xT / wqkv DMAs are paired so the first projection matmul starts ~6us in.
- wq/wk columns are permuted on the host (per-head de-interleave of RoPE
  pairs) making RoPE contiguous-block bf16 elementwise ops.
- The per-head qrot shuffle runs as 2 permutation matmuls per tile on the
  PE (cheaper than 16 strided DVE copies).
- Scores are computed transposed, ST[k, t] = K_rot @ Q_rot^T, so the AV
  matmul consumes exp(ST) directly with V in natural [t, d] layout and a
  ones-column in V yields the softmax denominators for free.
- Causal-mask (triu) multiplies run on the otherwise idle GpSimd engine.
- Normalization is split: reciprocal prep (DVE) emits right after the AV
  chain, the broadcast matmul + multiplies hide under the next head
  pair's score matmuls; po PSUM tiles are allocated late so pool slot
  rotation stays honest.
- Separate PSUM pools for scores / AV accumulators / projections avoid
  head-of-line blocking between phases.
- Phase A (proj+RoPE) chunks interleave with attention chunks to keep the
  PE continuously busy (p-state at full clock).
"""
import sys

sys.path.insert(0, "/opt/trn_rl_repo")
import ml_dtypes
import numpy as np

import concourse.bass as bass  # noqa: F401
import concourse.tile as tile
from concourse import bacc, mybir
from concourse.bass_utils import run_bass_kernel_spmd

F32 = mybir.dt.float32
BF16 = mybir.dt.bfloat16

B, T, DIM = 2, 2048, 1024
H, KV, HD = 16, 4, 64
NQ = H // KV          # q heads per core
THETA = 10000.0
SCALE = HD ** -0.5
NCORES = 8
QCH = 512             # q-chunk (free dim of scores/AV matmuls)
NQC = T // QCH        # 4 q-chunks
NKT = T // 128        # 16 k-tiles
XCH = 512
NCH = T // XCH        # 4 x-chunks
NKPC = 4              # k-tiles per chunk


def build_nc():
    nc = bacc.Bacc(None, target_bir_lowering=False)
    xT_d = nc.declare_dram_parameter("xT", [DIM, T], BF16, isOutput=False)
    wqkv_d = nc.declare_dram_parameter("wqkv", [DIM, 384], BF16, isOutput=False)
    wo_d = nc.declare_dram_parameter("wo", [256, DIM], BF16, isOutput=False)
    cosq_d = nc.declare_dram_parameter("cosq", [128, T], BF16, isOutput=False)
    sinq_d = nc.declare_dram_parameter("sinq", [128, T], BF16, isOutput=False)
    triu_d = nc.declare_dram_parameter("triu", [128, 128], BF16, isOutput=False)
    id_d = nc.declare_dram_parameter("ident", [128, 128], BF16, isOutput=False)
    perm_d = nc.declare_dram_parameter("perm", [128, 4, 128], BF16, isOutput=False)
    out_d = nc.declare_dram_parameter("out", [T, DIM], BF16, isOutput=True)

    with tile.TileContext(nc) as tc:
        with (
            tc.tile_pool(name="persist", bufs=1) as pp,
            tc.tile_pool(name="vpool", bufs=16) as vp,
            tc.tile_pool(name="chunk", bufs=2) as pch,
            tc.tile_pool(name="attn", bufs=4) as pb,
            tc.tile_pool(name="small", bufs=2) as pb2,
            tc.tile_pool(name="yst", bufs=6) as pys,
            tc.tile_pool(name="psS", bufs=2, space="PSUM") as psS,
            tc.tile_pool(name="psO", bufs=2, space="PSUM") as psO,
            tc.tile_pool(name="psP", bufs=2, space="PSUM") as psP,
        ):
            # ---- persistent tiles ----
            xt = {}
            for c in range(NCH):
                for d in range(8):
                    t = pp.tile([128, XCH], BF16, tag=f"xt{c}_{d}", name=f"xt{c}_{d}")
                    xt[(c, d)] = t
            wqkv_s = pp.tile([128, 8, 384], BF16, tag="wqkv_s")
            cosq = pp.tile([128, T], BF16, tag="cosq")
            sinq = pp.tile([128, T], BF16, tag="sinq")
            triu = pp.tile([128, 128], BF16, tag="triu")
            id_s = pp.tile([128, 128], BF16, tag="ident")
            perm = pp.tile([128, 4, 128], BF16, tag="perm")
            wo_s = pp.tile([128, 2, DIM], BF16, tag="wo_s")

            # ---- DMA preload: pair xT(c0) with wqkv so proj k-step data
            # arrives just in time; everything issues from the sync queue.
            wqkv_r = wqkv_d.rearrange("(k p) c -> p k c", p=128)
            for k in range(8):
                nc.sync.dma_start(xt[(0, k)][:], xT_d[k * 128 : (k + 1) * 128, 0:XCH])
                nc.scalar.dma_start(wqkv_s[:, k, :], wqkv_r[:, k, :])
            nc.scalar.dma_start(cosq[:, 0:XCH], cosq_d[:, 0:XCH])
            nc.scalar.dma_start(sinq[:, 0:XCH], sinq_d[:, 0:XCH])
            nc.scalar.dma_start(perm[:], perm_d[:])
            nc.scalar.dma_start(id_s[:], id_d[:])
            nc.scalar.dma_start(triu[:], triu_d[:])
            for c in range(1, NCH):
                for d in range(8):
                    nc.sync.dma_start(
                        xt[(c, d)][:],
                        xT_d[d * 128 : (d + 1) * 128, c * XCH : (c + 1) * XCH],
                    )
                nc.sync.dma_start(
                    cosq[:, c * XCH : (c + 1) * XCH],
                    cosq_d[:, c * XCH : (c + 1) * XCH],
                )
                nc.sync.dma_start(
                    sinq[:, c * XCH : (c + 1) * XCH],
                    sinq_d[:, c * XCH : (c + 1) * XCH],
                )
            wo_r = wo_d.rearrange("(k p) c -> p k c", p=128)
            for k in range(2):
                nc.sync.dma_start(wo_s[:, k, 0:512], wo_r[:, k, 0:512])
                nc.sync.dma_start(wo_s[:, k, 512:1024], wo_r[:, k, 512:1024])

            # persistent compute tiles
            qrot = [
                [
                    pp.tile([128, XCH], BF16, tag=f"qr{hp}_{c}", name=f"qr{hp}_{c}")
                    for c in range(NCH)
                ]
                for hp in range(2)
            ]
            krotc = [
                pp.tile([128, XCH], BF16, tag=f"kr{c}", name=f"kr{c}")
                for c in range(NCH)
            ]
            outTn = [
                [
                    pp.tile([128, XCH], BF16, tag=f"oT{hp}_{c}", name=f"oT{hp}_{c}")
                    for c in range(NCH)
                ]
                for hp in range(2)
            ]
            v_tiles = [
                vp.tile([128, HD + 1], BF16, tag="v", name=f"v{i}") for i in range(NKT)
            ]
            # ones33: selector for the denominator-broadcast matmul.
            ones33 = pp.tile([33, 128], BF16, tag="ones33")
            nc.gpsimd.memset(ones33[:], 0.0)
            nc.gpsimd.memset(ones33[0:1, 0:64], 1.0)
            nc.gpsimd.memset(ones33[32:33, 64:128], 1.0)
            for i in range(NKT):
                nc.gpsimd.memset(v_tiles[i][:, HD : HD + 1], 1.0)
            # pre-fill rotating den slots with 1.0 so rows 1..31 (never
            # written) stay finite for the reciprocal / broadcast matmul.
            for _ in range(2):
                t = pb2.tile([33, QCH], F32, tag="den", name="den_init")
                nc.gpsimd.memset(t[:], 1.0)

            # PE warmup: dummy matmuls while the first DMAs land, so the
            # p-state ramp (needs ~3us continuous busy) completes before
            # real work and LDWEIGHTS run at full clock.
            warm = pp.tile([128, 512], BF16, tag="warm")
            nc.vector.memset(warm[:], 0.0)
            wps = psP.tile([128, 512], F32, tag="pq", name="wps")
            for _ in range(14):
                nc.tensor.matmul(
                    wps[:], warm[:, 0:128], warm[:], start=True, stop=True
                )

            # ---- phase A: per-chunk projection + rope + v build ----
            def phase_a(nch):
                cs = slice(nch * XCH, (nch + 1) * XCH)
                ccs = cosq[:, cs]
                scs = sinq[:, cs]
                t0c = pch.tile([128, XCH], BF16, tag="t0c", name="t0c")
                t1c = pch.tile([128, XCH], BF16, tag="t1c", name="t1c")
                m2c = pch.tile([64, XCH], BF16, tag="m2c", name="m2c")
                vTc = pch.tile([64, XCH], BF16, tag="vTc", name="vTc")
                for m in range(3):
                    pq = psP.tile([128, XCH], F32, tag="pq", name="pq")
                    for k in range(8):
                        nc.tensor.matmul(
                            pq[:],
                            wqkv_s[:, k, m * 128 : (m + 1) * 128],
                            xt[(nch, k)][:],
                            start=(k == 0),
                            stop=(k == 7),
                        )
                    if m == 0:
                        nc.scalar.copy(t0c[:], pq[:])
                    elif m == 1:
                        nc.scalar.copy(t1c[:], pq[:])
                    else:
                        nc.vector.tensor_copy(m2c[:], pq[0:64, :])
                        nc.vector.tensor_copy(vTc[:], pq[64:128, :])

                # RoPE (q): sA = evens_rot, sB = odds_rot (bf16 throughout)
                sA = pch.tile([128, XCH], BF16, tag="sA", name="sA")
                sB = pch.tile([128, XCH], BF16, tag="sB", name="sB")
                tmp = pch.tile([128, XCH], BF16, tag="tmp", name="tmp")
                nc.vector.tensor_mul(sA[:], t0c[:], ccs)
                nc.vector.tensor_mul(tmp[:], t1c[:], scs)
                nc.vector.tensor_sub(sA[:], sA[:], tmp[:])
                nc.vector.tensor_mul(sB[:], t0c[:], scs)
                nc.vector.tensor_mul(tmp[:], t1c[:], ccs)
                nc.vector.tensor_add(sB[:], sB[:], tmp[:])

                # shuffle into per-head-pair layout via PE permutation matmuls
                for hp in range(2):
                    psq = psP.tile([128, XCH], F32, tag="pq", name="psq")
                    nc.tensor.matmul(
                        psq[:], perm[:, 2 * hp, :], sA[:], start=True, stop=False
                    )
                    nc.tensor.matmul(
                        psq[:], perm[:, 2 * hp + 1, :], sB[:], start=False, stop=True
                    )
                    nc.scalar.copy(qrot[hp][nch][:], psq[:])

                # RoPE (k) on m2c rows [ke(32); ko(32)], duplicated rows 64:128
                krc = krotc[nch]
                k1 = pch.tile([32, XCH], BF16, tag="k1", name="k1")
                k2 = pch.tile([32, XCH], BF16, tag="k2", name="k2")
                k3 = pch.tile([32, XCH], BF16, tag="k3", name="k3")
                k4 = pch.tile([32, XCH], BF16, tag="k4", name="k4")
                nc.gpsimd.tensor_mul(k1[:], m2c[0:32, :], ccs[0:32, :])
                nc.gpsimd.tensor_mul(k2[:], m2c[32:64, :], scs[32:64, :])
                nc.gpsimd.tensor_mul(k3[:], m2c[0:32, :], scs[0:32, :])
                nc.gpsimd.tensor_mul(k4[:], m2c[32:64, :], ccs[32:64, :])
                nc.vector.tensor_sub(krc[0:32, :], k1[:], k2[:])
                nc.vector.tensor_add(krc[32:64, :], k3[:], k4[:])
                nc.gpsimd.tensor_copy(krc[64:128, :], krc[0:64, :])

                # V tiles for this chunk (transpose v^T [64, t] -> [t, 64])
                for ii in range(NKPC):
                    i = nch * NKPC + ii
                    pv = psP.tile([128, HD], BF16, tag="pq", name="pv")
                    nc.tensor.transpose(
                        pv[:],
                        vTc[:, ii * 128 : (ii + 1) * 128],
                        id_s[0:64, 0:64],
                    )
                    nc.vector.tensor_copy(v_tiles[i][:, 0:HD], pv[:])

            # ---- phase B: attention with split normalization ----
            def attention_chunk(qc):
                nkt = NKPC * (qc + 1)
                pending_post = [None]

                def norm_pre(hp, po):
                    den = pb2.tile([33, QCH], F32, tag="den", name="den")
                    nc.vector.tensor_copy(den[0:1, :], po[0][HD : HD + 1, :])
                    nc.vector.tensor_copy(den[32:33, :], po[1][HD : HD + 1, :])
                    invf = pb2.tile([33, QCH], F32, tag="invf", name="invf")
                    nc.vector.reciprocal_approx_fast(invf[:], den[:])
                    invb = pb2.tile([33, QCH], BF16, tag="invb", name="invb")
                    nc.vector.tensor_copy(invb[:], invf[:])
                    return invb

                def norm_post(hp, po, invb):
                    bc = psP.tile([128, QCH], F32, tag="pq", name="bc")
                    nc.tensor.matmul(bc[:], ones33[:], invb[:], start=True, stop=True)
                    bcS = pb2.tile([128, QCH], BF16, tag="bcS", name="bcS")
                    nc.vector.tensor_copy(bcS[:], bc[:])
                    for i in range(2):
                        nc.vector.tensor_mul(
                            outTn[hp][qc][64 * i : 64 * i + 64, :],
                            po[i][0:HD, :],
                            bcS[64 * i : 64 * i + 64, :],
                        )

                def flush_post():
                    if pending_post[0] is not None:
                        norm_post(*pending_post[0])
                        pending_post[0] = None

                for hp in range(2):
                    po = [None, None]

                    def emit_avs(entry):
                        ktv, col0, et2 = entry
                        if po[0] is None:
                            # late alloc: all readers of the previous slot
                            # tiles are already emitted at this point
                            po[0] = psO.tile([HD + 1, QCH], F32, tag="po", name="po0")
                            po[1] = psO.tile([HD + 1, QCH], F32, tag="po", name="po1")
                        nc.tensor.matmul(
                            po[0][:, col0:QCH],
                            v_tiles[ktv][:],
                            et2[:, col0:QCH],
                            start=(ktv == 0),
                            stop=(ktv == nkt - 1),
                        )
                        nc.tensor.matmul(
                            po[1][:, col0:QCH],
                            v_tiles[ktv][:],
                            et2[:, QCH : 2 * QCH - col0],
                            start=(ktv == 0),
                            stop=(ktv == nkt - 1),
                        )

                    pend = []
                    for kt in range(nkt):
                        j = kt - NKPC * qc
                        col0 = 128 * j if j >= 0 else 0
                        kr = krotc[kt // NKPC]
                        kslice = slice((kt % NKPC) * 128, (kt % NKPC) * 128 + 128)
                        qt = qrot[hp][qc]
                        tslice = slice(col0, QCH)
                        ps = psS.tile([128, 2 * QCH], F32, tag="ps", name="ps")
                        nc.tensor.matmul(
                            ps[:, col0:QCH],
                            kr[0:64, kslice],
                            qt[0:64, tslice],
                            start=True,
                            stop=True,
                            tile_position=(0, 0),
                        )
                        nc.tensor.matmul(
                            ps[:, QCH : 2 * QCH - col0],
                            kr[64:128, kslice],
                            qt[64:128, tslice],
                            start=True,
                            stop=True,
                            tile_position=(64, 0),
                        )
                        et2 = pb.tile([128, 2 * QCH], BF16, tag="et", name="et")
                        nc.scalar.activation(
                            et2[:, col0 : 2 * QCH - col0],
                            ps[:, col0 : 2 * QCH - col0],
                            mybir.ActivationFunctionType.Exp,
                            scale=SCALE,
                        )
                        if j >= 0:
                            nc.gpsimd.tensor_mul(
                                et2[:, col0 : col0 + 128],
                                et2[:, col0 : col0 + 128],
                                triu[:],
                            )
                            nc.gpsimd.tensor_mul(
                                et2[:, QCH : QCH + 128],
                                et2[:, QCH : QCH + 128],
                                triu[:],
                            )
                        pend.append((kt, col0, et2))
                        if len(pend) > 1:
                            # previous head pair's bc/muls must be emitted
                            # before our late po alloc (slot reuse), and they
                            # hide under our first score matmuls
                            flush_post()
                            emit_avs(pend.pop(0))
                    for e in pend:
                        emit_avs(e)
                    flush_post()
                    invb = norm_pre(hp, po)
                    pending_post[0] = (hp, po, invb)
                return flush_post

            def proj_y(qc, last=False):
                for tq in range(qc * NKPC, (qc + 1) * NKPC):
                    ksl = slice((tq % NKPC) * 128, (tq % NKPC) * 128 + 128)
                    for n2 in range(2):
                        py = psP.tile([128, 512], F32, tag="pq", name="py")
                        nc.tensor.matmul(
                            py[:],
                            outTn[0][qc][:, ksl],
                            wo_s[:, 0, n2 * 512 : (n2 + 1) * 512],
                            start=True,
                            stop=False,
                        )
                        nc.tensor.matmul(
                            py[:],
                            outTn[1][qc][:, ksl],
                            wo_s[:, 1, n2 * 512 : (n2 + 1) * 512],
                            start=False,
                            stop=True,
                        )
                        ys = pys.tile([128, 512], BF16, tag="ys", name="ys")
                        if last or n2 == 0:
                            eng, dma_eng = nc.vector.tensor_copy, nc.sync
                        else:
                            eng, dma_eng = nc.scalar.copy, nc.scalar
                        eng(ys[:], py[:])
                        orow = out_d[tq * 128 : (tq + 1) * 128, :]
                        if last:
                            # halve the trailing write so the drain is shorter
                            dma_eng.dma_start(
                                orow[:, n2 * 512 : n2 * 512 + 256], ys[:, 0:256]
                            )
                            dma_eng.dma_start(
                                orow[:, n2 * 512 + 256 : (n2 + 1) * 512], ys[:, 256:512]
                            )
                        else:
                            dma_eng.dma_start(
                                orow[:, n2 * 512 : (n2 + 1) * 512], ys[:]
                            )

            # ---- interleaved schedule ----
            phase_a(0)
            phase_a(1)
            f1 = attention_chunk(1)
            phase_a(2)
            f1()
            proj_y(1)
            f2 = attention_chunk(2)
            phase_a(3)
            f2()
            proj_y(2)
            f3 = attention_chunk(3)
            f3()
            proj_y(3)
            f0 = attention_chunk(0)
            f0()
            proj_y(0, last=True)
    nc.compile()
    return nc


def _host_tables():
    ev = np.arange(0, HD, 2)
    od = ev + 1
    inv = 1.0 / (THETA ** (np.arange(0, HD, 2, dtype=np.float64) / HD))  # [32]
    freqs = np.outer(inv, np.arange(T, dtype=np.float64))  # [32, T]
    cosq = np.tile(np.cos(freqs), (4, 1)).astype(ml_dtypes.bfloat16)  # [128, T]
    sinq = np.tile(np.sin(freqs), (4, 1)).astype(ml_dtypes.bfloat16)
    triu = np.triu(np.ones((128, 128), np.float32)).astype(ml_dtypes.bfloat16)
    ident = np.eye(128, dtype=np.float32).astype(ml_dtypes.bfloat16)
    # permutation matrices for the qrot shuffle: qrot[hp] = PA^T sA + PB^T sB
    # sA rows 32h:32h+32 = head h rotated evens; sB same rows = odds.
    # dest qrot[hp]: rows 0:32 = head 2hp evens, 32:64 = head 2hp odds,
    #                64:96 = head 2hp+1 evens, 96:128 = head 2hp+1 odds.
    perm = np.zeros((128, 4, 128), np.float32)
    for hp in range(2):
        pa = perm[:, 2 * hp, :]
        pbm = perm[:, 2 * hp + 1, :]
        for s in range(32):
            pa[32 * (2 * hp) + s, s] = 1.0
            pa[32 * (2 * hp + 1) + s, 64 + s] = 1.0
            pbm[32 * (2 * hp) + s, 32 + s] = 1.0
            pbm[32 * (2 * hp + 1) + s, 96 + s] = 1.0
    perm = perm.astype(ml_dtypes.bfloat16)
    return ev, od, cosq, sinq, triu, ident, perm


def make_in_maps(inputs):
    x = np.asarray(inputs["x"], dtype=np.float32)
    wq = np.asarray(inputs["wq"], dtype=np.float32)
    wk = np.asarray(inputs["wk"], dtype=np.float32)
    wv = np.asarray(inputs["wv"], dtype=np.float32)
    wo = np.asarray(inputs["wo"], dtype=np.float32)
    ev, od, cosq, sinq, triu, ident, perm = _host_tables()
    xT = [np.ascontiguousarray(x[b].T).astype(ml_dtypes.bfloat16) for b in range(B)]
    in_maps = []
    for c in range(NCORES):
        b, g = c // 4, c % 4
        qe = np.concatenate([wq[:, 64 * (4 * g + h) + ev] for h in range(NQ)], axis=1)
        qo = np.concatenate([wq[:, 64 * (4 * g + h) + od] for h in range(NQ)], axis=1)
        wqkv_g = np.concatenate(
            [
                qe,
                qo,
                wk[:, 64 * g + ev],
                wk[:, 64 * g + od],
                wv[:, 64 * g : 64 * (g + 1)],
            ],
            axis=1,
        ).astype(ml_dtypes.bfloat16)  # [1024, 384]
        wo_g = wo[256 * g : 256 * (g + 1), :].astype(ml_dtypes.bfloat16)
        in_maps.append(
            {
                "xT": xT[b],
                "wqkv": np.ascontiguousarray(wqkv_g),
                "wo": np.ascontiguousarray(wo_g),
                "cosq": cosq,
                "sinq": sinq,
                "triu": triu,
                "ident": ident,
                "perm": perm,
            }
        )
    return in_maps


_NC_CACHE = None


def kernel(**inputs):
    global _NC_CACHE
    if _NC_CACHE is None:
        _NC_CACHE = build_nc()
    in_maps = make_in_maps(inputs)
    res = run_bass_kernel_spmd(_NC_CACHE, in_maps, list(range(NCORES)))
    out = np.zeros((B, T, DIM), np.float32)
    for c in range(NCORES):
        out[c // 4] += np.asarray(res.results[c]["out"], dtype=np.float32)
    return out


# revision 23
# speedup vs baseline: 1.2951x; 1.2951x over previous
"""GQA attention (RoPE, causal) on 8 TRN2 NeuronCores.

Sharding: core c = (b, g) with b = c // 4 (batch), g = c % 4 (kv-group).
Each core computes 4 query heads + 1 kv head of attention for one batch
element, plus its slice of the output projection; the host sums the 4
partial outputs per batch (row-parallel wo unshard).

Layout tricks:
- x is shipped pre-transposed (xT [DIM, T]) so no on-device transposes;
  xT / wqkv DMAs are paired so the first projection matmul starts ~6us in.
- wq/wk columns are permuted on the host (per-head de-interleave of RoPE
  pairs) making RoPE contiguous-block bf16 elementwise ops.
- The per-head qrot shuffle runs as 2 permutation matmuls per tile on the
  PE (cheaper than 16 strided DVE copies).
- Scores are computed transposed, ST[k, t] = K_rot @ Q_rot^T, so the AV
  matmul consumes exp(ST) directly with V in natural [t, d] layout and a
  ones-column in V yields the softmax denominators for free.
- Causal-mask (triu) multiplies run on the otherwise idle GpSimd engine.
- Normalization is split: reciprocal prep (DVE) emits right after the AV
  chain, the broadcast matmul + multiplies hide under the next head
  pair's score matmuls; po PSUM tiles are allocated late so pool slot
  rotation stays honest.
- Separate PSUM pools for scores / AV accumulators / projections avoid
  head-of-line blocking between phases.
- Phase A (proj+RoPE) chunks interleave with attention chunks to keep the
  PE continuously busy (p-state at full clock).
"""
import sys

sys.path.insert(0, "/opt/trn_rl_repo")
import ml_dtypes
import numpy as np

import concourse.bass as bass  # noqa: F401
import concourse.tile as tile
from concourse import bacc, mybir
from concourse.bass_utils import run_bass_kernel_spmd

F32 = mybir.dt.float32
BF16 = mybir.dt.bfloat16

B, T, DIM = 2, 2048, 1024
H, KV, HD = 16, 4, 64
NQ = H // KV          # q heads per core
THETA = 10000.0
SCALE = HD ** -0.5
NCORES = 8
QCH = 512             # q-chunk (free dim of scores/AV matmuls)
NQC = T // QCH        # 4 q-chunks
NKT = T // 128        # 16 k-tiles
XCH = 512
NCH = T // XCH        # 4 x-chunks
NKPC = 4              # k-tiles per chunk


def build_nc():
    nc = bacc.Bacc(None, target_bir_lowering=False)
    xT_d = nc.declare_dram_parameter("xT", [DIM, T], BF16, isOutput=False)
    wqkv_d = nc.declare_dram_parameter("wqkv", [DIM, 384], BF16, isOutput=False)
    wo_d = nc.declare_dram_parameter("wo", [256, DIM], BF16, isOutput=False)
    cosq_d = nc.declare_dram_parameter("cosq", [128, T], BF16, isOutput=False)
    sinq_d = nc.declare_dram_parameter("sinq", [128, T], BF16, isOutput=False)
    triu_d = nc.declare_dram_parameter("triu", [128, 128], BF16, isOutput=False)
    id_d = nc.declare_dram_parameter("ident", [128, 128], BF16, isOutput=False)
    perm_d = nc.declare_dram_parameter("perm", [128, 4, 128], BF16, isOutput=False)
    out_d = nc.declare_dram_parameter("out", [T, DIM], BF16, isOutput=True)

    with tile.TileContext(nc) as tc:
        with (
            tc.tile_pool(name="persist", bufs=1) as pp,
            tc.tile_pool(name="vpool", bufs=16) as vp,
            tc.tile_pool(name="chunk", bufs=2) as pch,
            tc.tile_pool(name="attn", bufs=4) as pb,
            tc.tile_pool(name="small", bufs=2) as pb2,
            tc.tile_pool(name="yst", bufs=6) as pys,
            tc.tile_pool(name="psS", bufs=2, space="PSUM") as psS,
            tc.tile_pool(name="psO", bufs=2, space="PSUM") as psO,
            tc.tile_pool(name="psP", bufs=2, space="PSUM") as psP,
        ):
            # ---- persistent tiles ----
            xt = {}
            for c in range(NCH):
                for d in range(8):
                    t = pp.tile([128, XCH], BF16, tag=f"xt{c}_{d}", name=f"xt{c}_{d}")
                    xt[(c, d)] = t
            wqkv_s = pp.tile([128, 8, 384], BF16, tag="wqkv_s")
            cosq = pp.tile([128, T], BF16, tag="cosq")
            sinq = pp.tile([128, T], BF16, tag="sinq")
            triu = pp.tile([128, 128], BF16, tag="triu")
            id_s = pp.tile([128, 128], BF16, tag="ident")
            perm = pp.tile([128, 4, 128], BF16, tag="perm")
            wo_s = pp.tile([128, 2, DIM], BF16, tag="wo_s")

            # ---- DMA preload: pair xT(c0) with wqkv so proj k-step data
            # arrives just in time; everything issues from the sync queue.
            wqkv_r = wqkv_d.rearrange("(k p) c -> p k c", p=128)
            for k in range(8):
                nc.sync.dma_start(xt[(0, k)][:], xT_d[k * 128 : (k + 1) * 128, 0:XCH])
                nc.scalar.dma_start(wqkv_s[:, k, :], wqkv_r[:, k, :])
            nc.scalar.dma_start(cosq[:, 0:XCH], cosq_d[:, 0:XCH])
            nc.scalar.dma_start(sinq[:, 0:XCH], sinq_d[:, 0:XCH])
            nc.scalar.dma_start(perm[:], perm_d[:])
            nc.scalar.dma_start(id_s[:], id_d[:])
            nc.scalar.dma_start(triu[:], triu_d[:])
            for c in range(1, NCH):
                for d in range(8):
                    nc.sync.dma_start(
                        xt[(c, d)][:],
                        xT_d[d * 128 : (d + 1) * 128, c * XCH : (c + 1) * XCH],
                    )
                nc.sync.dma_start(
                    cosq[:, c * XCH : (c + 1) * XCH],
                    cosq_d[:, c * XCH : (c + 1) * XCH],
                )
                nc.sync.dma_start(
                    sinq[:, c * XCH : (c + 1) * XCH],
                    sinq_d[:, c * XCH : (c + 1) * XCH],
                )
            wo_r = wo_d.rearrange("(k p) c -> p k c", p=128)
            for k in range(2):
                nc.sync.dma_start(wo_s[:, k, 0:512], wo_r[:, k, 0:512])
                nc.sync.dma_start(wo_s[:, k, 512:1024], wo_r[:, k, 512:1024])

            # persistent compute tiles
            qrot = [
                [
                    pp.tile([128, XCH], BF16, tag=f"qr{hp}_{c}", name=f"qr{hp}_{c}")
                    for c in range(NCH)
                ]
                for hp in range(2)
            ]
            krotc = [
                pp.tile([128, XCH], BF16, tag=f"kr{c}", name=f"kr{c}")
                for c in range(NCH)
            ]
            outTn = [
                [
                    pp.tile([128, XCH], BF16, tag=f"oT{hp}_{c}", name=f"oT{hp}_{c}")
                    for c in range(NCH)
                ]
                for hp in range(2)
            ]
            v_tiles = [
                vp.tile([128, HD + 1], BF16, tag="v", name=f"v{i}") for i in range(NKT)
            ]
            # ones33: selector for the denominator-broadcast matmul.
            ones33 = pp.tile([33, 128], BF16, tag="ones33")
            nc.gpsimd.memset(ones33[:], 0.0)
            nc.gpsimd.memset(ones33[0:1, 0:64], 1.0)
            nc.gpsimd.memset(ones33[32:33, 64:128], 1.0)
            for i in range(NKT):
                nc.gpsimd.memset(v_tiles[i][:, HD : HD + 1], 1.0)
            # pre-fill rotating den slots with 1.0 so rows 1..31 (never
            # written) stay finite for the reciprocal / broadcast matmul.
            for _ in range(2):
                t = pb2.tile([33, QCH], F32, tag="den", name="den_init")
                nc.gpsimd.memset(t[:], 1.0)

            # PE warmup: dummy matmuls while the first DMAs land, so the
            # p-state ramp (needs ~3us continuous busy) completes before
            # real work and LDWEIGHTS run at full clock.
            warm = pp.tile([128, 512], BF16, tag="warm")
            nc.vector.memset(warm[:], 0.0)
            wps = psP.tile([128, 512], F32, tag="pq", name="wps")
            for _ in range(14):
                nc.tensor.matmul(
                    wps[:], warm[:, 0:128], warm[:], start=True, stop=True
                )

            # ---- phase A: per-chunk projection + rope + v build ----
            def phase_a(nch):
                cs = slice(nch * XCH, (nch + 1) * XCH)
                ccs = cosq[:, cs]
                scs = sinq[:, cs]
                t0c = pch.tile([128, XCH], BF16, tag="t0c", name="t0c")
                t1c = pch.tile([128, XCH], BF16, tag="t1c", name="t1c")
                m2c = pch.tile([64, XCH], BF16, tag="m2c", name="m2c")
                vTc = pch.tile([64, XCH], BF16, tag="vTc", name="vTc")
                for m in range(3):
                    pq = psP.tile([128, XCH], F32, tag="pq", name="pq")
                    for k in range(8):
                        nc.tensor.matmul(
                            pq[:],
                            wqkv_s[:, k, m * 128 : (m + 1) * 128],
                            xt[(nch, k)][:],
                            start=(k == 0),
                            stop=(k == 7),
                        )
                    if m == 0:
                        nc.scalar.copy(t0c[:], pq[:])
                    elif m == 1:
                        nc.scalar.copy(t1c[:], pq[:])
                    else:
                        nc.vector.tensor_copy(m2c[:], pq[0:64, :])
                        nc.vector.tensor_copy(vTc[:], pq[64:128, :])

                # RoPE (q): sA = evens_rot, sB = odds_rot (bf16 throughout)
                sA = pch.tile([128, XCH], BF16, tag="sA", name="sA")
                sB = pch.tile([128, XCH], BF16, tag="sB", name="sB")
                tmp = pch.tile([128, XCH], BF16, tag="tmp", name="tmp")
                nc.vector.tensor_mul(sA[:], t0c[:], ccs)
                nc.vector.tensor_mul(tmp[:], t1c[:], scs)
                nc.vector.tensor_sub(sA[:], sA[:], tmp[:])
                nc.vector.tensor_mul(sB[:], t0c[:], scs)
                nc.vector.tensor_mul(tmp[:], t1c[:], ccs)
                nc.vector.tensor_add(sB[:], sB[:], tmp[:])

                # shuffle into per-head-pair layout via PE permutation matmuls
                for hp in range(2):
                    psq = psP.tile([128, XCH], F32, tag="pq", name="psq")
                    nc.tensor.matmul(
                        psq[:], perm[:, 2 * hp, :], sA[:], start=True, stop=False
                    )
                    nc.tensor.matmul(
                        psq[:], perm[:, 2 * hp + 1, :], sB[:], start=False, stop=True
                    )
                    nc.scalar.copy(qrot[hp][nch][:], psq[:])

                # RoPE (k) on m2c rows [ke(32); ko(32)], duplicated rows 64:128
                krc = krotc[nch]
                k1 = pch.tile([32, XCH], BF16, tag="k1", name="k1")
                k2 = pch.tile([32, XCH], BF16, tag="k2", name="k2")
                nc.vector.tensor_mul(k1[:], m2c[0:32, :], ccs[0:32, :])
                nc.vector.tensor_mul(k2[:], m2c[32:64, :], scs[32:64, :])
                nc.vector.tensor_sub(krc[0:32, :], k1[:], k2[:])
                nc.vector.tensor_mul(k1[:], m2c[0:32, :], scs[0:32, :])
                nc.vector.tensor_mul(k2[:], m2c[32:64, :], ccs[32:64, :])
                nc.vector.tensor_add(krc[32:64, :], k1[:], k2[:])
                nc.gpsimd.tensor_copy(krc[64:128, :], krc[0:64, :])

                # V tiles for this chunk (transpose v^T [64, t] -> [t, 64])
                for ii in range(NKPC):
                    i = nch * NKPC + ii
                    pv = psP.tile([128, HD], BF16, tag="pq", name="pv")
                    nc.tensor.transpose(
                        pv[:],
                        vTc[:, ii * 128 : (ii + 1) * 128],
                        id_s[0:64, 0:64],
                    )
                    nc.vector.tensor_copy(v_tiles[i][:, 0:HD], pv[:])

            # ---- phase B: attention with split normalization ----
            def attention_chunk(qc):
                nkt = NKPC * (qc + 1)
                pending_post = [None]

                def norm_pre(hp, po):
                    den = pb2.tile([33, QCH], F32, tag="den", name="den")
                    nc.vector.tensor_copy(den[0:1, :], po[0][HD : HD + 1, :])
                    nc.vector.tensor_copy(den[32:33, :], po[1][HD : HD + 1, :])
                    invf = pb2.tile([33, QCH], F32, tag="invf", name="invf")
                    nc.vector.reciprocal_approx_fast(invf[:], den[:])
                    invb = pb2.tile([33, QCH], BF16, tag="invb", name="invb")
                    nc.vector.tensor_copy(invb[:], invf[:])
                    return invb

                def norm_post(hp, po, invb):
                    bc = psP.tile([128, QCH], F32, tag="pq", name="bc")
                    nc.tensor.matmul(bc[:], ones33[:], invb[:], start=True, stop=True)
                    bcS = pb2.tile([128, QCH], BF16, tag="bcS", name="bcS")
                    nc.vector.tensor_copy(bcS[:], bc[:])
                    for i in range(2):
                        nc.vector.tensor_mul(
                            outTn[hp][qc][64 * i : 64 * i + 64, :],
                            po[i][0:HD, :],
                            bcS[64 * i : 64 * i + 64, :],
                        )

                def flush_post():
                    if pending_post[0] is not None:
                        norm_post(*pending_post[0])
                        pending_post[0] = None

                for hp in range(2):
                    po = [None, None]

                    def emit_avs(entry):
                        ktv, col0, et2 = entry
                        if po[0] is None:
                            # late alloc: all readers of the previous slot
                            # tiles are already emitted at this point
                            po[0] = psO.tile([HD + 1, QCH], F32, tag="po", name="po0")
                            po[1] = psO.tile([HD + 1, QCH], F32, tag="po", name="po1")
                        nc.tensor.matmul(
                            po[0][:, col0:QCH],
                            v_tiles[ktv][:],
                            et2[:, col0:QCH],
                            start=(ktv == 0),
                            stop=(ktv == nkt - 1),
                        )
                        nc.tensor.matmul(
                            po[1][:, col0:QCH],
                            v_tiles[ktv][:],
                            et2[:, QCH : 2 * QCH - col0],
                            start=(ktv == 0),
                            stop=(ktv == nkt - 1),
                        )

                    pend = []
                    for kt in range(nkt):
                        j = kt - NKPC * qc
                        col0 = 128 * j if j >= 0 else 0
                        kr = krotc[kt // NKPC]
                        kslice = slice((kt % NKPC) * 128, (kt % NKPC) * 128 + 128)
                        qt = qrot[hp][qc]
                        tslice = slice(col0, QCH)
                        ps = psS.tile([128, 2 * QCH], F32, tag="ps", name="ps")
                        nc.tensor.matmul(
                            ps[:, col0:QCH],
                            kr[0:64, kslice],
                            qt[0:64, tslice],
                            start=True,
                            stop=True,
                            tile_position=(0, 0),
                        )
                        nc.tensor.matmul(
                            ps[:, QCH : 2 * QCH - col0],
                            kr[64:128, kslice],
                            qt[64:128, tslice],
                            start=True,
                            stop=True,
                            tile_position=(64, 0),
                        )
                        et2 = pb.tile([128, 2 * QCH], BF16, tag="et", name="et")
                        nc.scalar.activation(
                            et2[:, col0 : 2 * QCH - col0],
                            ps[:, col0 : 2 * QCH - col0],
                            mybir.ActivationFunctionType.Exp,
                            scale=SCALE,
                        )
                        if j >= 0:
                            nc.gpsimd.tensor_mul(
                                et2[:, col0 : col0 + 128],
                                et2[:, col0 : col0 + 128],
                                triu[:],
                            )
                            nc.gpsimd.tensor_mul(
                                et2[:, QCH : QCH + 128],
                                et2[:, QCH : QCH + 128],
                                triu[:],
                            )
                        pend.append((kt, col0, et2))
                        if len(pend) > 1:
                            # previous head pair's bc/muls must be emitted
                            # before our late po alloc (slot reuse), and they
                            # hide under our first score matmuls
                            flush_post()
                            emit_avs(pend.pop(0))
                    for e in pend:
                        emit_avs(e)
                    flush_post()
                    invb = norm_pre(hp, po)
                    pending_post[0] = (hp, po, invb)
                return flush_post

            def proj_y(qc, last=False):
                for tq in range(qc * NKPC, (qc + 1) * NKPC):
                    ksl = slice((tq % NKPC) * 128, (tq % NKPC) * 128 + 128)
                    for n2 in range(2):
                        py = psP.tile([128, 512], F32, tag="pq", name="py")
                        nc.tensor.matmul(
                            py[:],
                            outTn[0][qc][:, ksl],
                            wo_s[:, 0, n2 * 512 : (n2 + 1) * 512],
                            start=True,
                            stop=False,
                        )
                        nc.tensor.matmul(
                            py[:],
                            outTn[1][qc][:, ksl],
                            wo_s[:, 1, n2 * 512 : (n2 + 1) * 512],
                            start=False,
                            stop=True,
                        )
                        ys = pys.tile([128, 512], BF16, tag="ys", name="ys")
                        if last or n2 == 0:
                            eng, dma_eng = nc.vector.tensor_copy, nc.sync
                        else:
                            eng, dma_eng = nc.scalar.copy, nc.scalar
                        eng(ys[:], py[:])
                        orow = out_d[tq * 128 : (tq + 1) * 128, :]
                        if last:
                            # halve the trailing write so the drain is shorter
                            dma_eng.dma_start(
                                orow[:, n2 * 512 : n2 * 512 + 256], ys[:, 0:256]
                            )
                            dma_eng.dma_start(
                                orow[:, n2 * 512 + 256 : (n2 + 1) * 512], ys[:, 256:512]
                            )
                        else:
                            dma_eng.dma_start(
                                orow[:, n2 * 512 : (n2 + 1) * 512], ys[:]
                            )

            # ---- interleaved schedule ----
            phase_a(0)
            phase_a(1)
            f1 = attention_chunk(1)
            phase_a(2)
            f1()
            proj_y(1)
            f2 = attention_chunk(2)
            phase_a(3)
            f2()
            proj_y(2)
            f3 = attention_chunk(3)
            f3()
            proj_y(3)
            f0 = attention_chunk(0)
            f0()
            proj_y(0, last=True)
    nc.compile()
    return nc


def _host_tables():
    ev = np.arange(0, HD, 2)
    od = ev + 1
    inv = 1.0 / (THETA ** (np.arange(0, HD, 2, dtype=np.float64) / HD))  # [32]
    freqs = np.outer(inv, np.arange(T, dtype=np.float64))  # [32, T]
    cosq = np.tile(np.cos(freqs), (4, 1)).astype(ml_dtypes.bfloat16)  # [128, T]
    sinq = np.tile(np.sin(freqs), (4, 1)).astype(ml_dtypes.bfloat16)
    triu = np.triu(np.ones((128, 128), np.float32)).astype(ml_dtypes.bfloat16)
    ident = np.eye(128, dtype=np.float32).astype(ml_dtypes.bfloat16)
    # permutation matrices for the qrot shuffle: qrot[hp] = PA^T sA + PB^T sB
    # sA rows 32h:32h+32 = head h rotated evens; sB same rows = odds.
    # dest qrot[hp]: rows 0:32 = head 2hp evens, 32:64 = head 2hp odds,
    #                64:96 = head 2hp+1 evens, 96:128 = head 2hp+1 odds.
    perm = np.zeros((128, 4, 128), np.float32)
    for hp in range(2):
        pa = perm[:, 2 * hp, :]
        pbm = perm[:, 2 * hp + 1, :]
        for s in range(32):
            pa[32 * (2 * hp) + s, s] = 1.0
            pa[32 * (2 * hp + 1) + s, 64 + s] = 1.0
            pbm[32 * (2 * hp) + s, 32 + s] = 1.0
            pbm[32 * (2 * hp + 1) + s, 96 + s] = 1.0
    perm = perm.astype(ml_dtypes.bfloat16)
    return ev, od, cosq, sinq, triu, ident, perm


def make_in_maps(inputs):
    x = np.asarray(inputs["x"], dtype=np.float32)
    wq = np.asarray(inputs["wq"], dtype=np.float32)
    wk = np.asarray(inputs["wk"], dtype=np.float32)
    wv = np.asarray(inputs["wv"], dtype=np.float32)
    wo = np.asarray(inputs["wo"], dtype=np.float32)
    ev, od, cosq, sinq, triu, ident, perm = _host_tables()
    xT = [np.ascontiguousarray(x[b].T).astype(ml_dtypes.bfloat16) for b in range(B)]
    in_maps = []
    for c in range(NCORES):
        b, g = c // 4, c % 4
        qe = np.concatenate([wq[:, 64 * (4 * g + h) + ev] for h in range(NQ)], axis=1)
        qo = np.concatenate([wq[:, 64 * (4 * g + h) + od] for h in range(NQ)], axis=1)
        wqkv_g = np.concatenate(
            [
                qe,
                qo,
                wk[:, 64 * g + ev],
                wk[:, 64 * g + od],
                wv[:, 64 * g : 64 * (g + 1)],
            ],
            axis=1,
        ).astype(ml_dtypes.bfloat16)  # [1024, 384]
        wo_g = wo[256 * g : 256 * (g + 1), :].astype(ml_dtypes.bfloat16)
        in_maps.append(
            {
                "xT": xT[b],
                "wqkv": np.ascontiguousarray(wqkv_g),
                "wo": np.ascontiguousarray(wo_g),
                "cosq": cosq,
                "sinq": sinq,
                "triu": triu,
                "ident": ident,
                "perm": perm,
            }
        )
    return in_maps


_NC_CACHE = None


def kernel(**inputs):
    global _NC_CACHE
    if _NC_CACHE is None:
        _NC_CACHE = build_nc()
    in_maps = make_in_maps(inputs)
    res = run_bass_kernel_spmd(_NC_CACHE, in_maps, list(range(NCORES)))
    out = np.zeros((B, T, DIM), np.float32)
    for c in range(NCORES):
        out[c // 4] += np.asarray(res.results[c]["out"], dtype=np.float32)
    return out
